# revision 14
# baseline (speedup 1.0000x reference)
"""GCN message-passing kernel for 8 Trainium2 NeuronCores.

Strategy: shard CHANNELS across the 8 cores (C=1280 -> 160 ch/core). Each core
computes the full output for its channel slice with zero collectives:
  - edge-encoder MLP: gamma/beta only for this core's 160 channels (W2 column
    shard), with the full h = relu(pose @ W1 + b1) recomputed per core on PE.
  - edges are host-sorted by dst and padded per 128-node dst window; the
    scatter-sum is a one-hot matmul on PE accumulating into PSUM per window.
  - the per-edge gather image[src] is an indirect DMA (4 blocks = 512 edges
    per issue to amortize SWDGE cost) from an HBM image slice [node, hw, ch].
  - mean = PSUM evacuation with per-partition scale 1/cnt (0 for empty nodes).

v2: software-pipelined with a 2-chunk skew so PE streams continuously:
  step i emits h-matmuls(chunk i), e-matmuls+sigmoid(i-1), multiplies(i-1),
  scatter matmuls(i-2). h-PSUM evacuation rotates over ACT/DVE/GPSIMD.
  Window evacuation is a single scalar_tensor_tensor pass (+ tiny beta scale).
"""

import sys

sys.path.insert(0, "/opt/trn_rl_repo")

import numpy as np
import ml_dtypes

import concourse.bass as bass
import concourse.mybir as mybir
from concourse.tile import TileContext
from concourse.bass_utils import run_bass_kernel_spmd


BF16 = ml_dtypes.bfloat16
FP8 = ml_dtypes.float8_e4m3
P = 128
N_CORES = 8
CH_EDGES = 512  # edges per pipeline chunk
BPC = CH_EDGES // P  # blocks per chunk


def _split_excess_waits(nc):
    """This walrus build only encodes 1 sem-wait per instruction; hoist extra
    waits onto same-engine NoOps placed just before (engines run in order)."""
    for bb in nc.main_func.blocks:
        new_insts = []
        for ins in bb.instructions:
            si = ins.sync_info
            limit = 1
            if si is not None and si.on_wait and len(si.on_wait) > limit:
                waits = list(si.on_wait)
                extra, keep = waits[:-limit], waits[-limit:]
                for k, w in enumerate(extra):
                    nop = mybir.InstNoOp(name=f"{ins.name}-ws-{k}", ins=[], outs=[])
                    nop.engine = ins.engine
                    nop.sync_info = mybir.SyncInfo(on_wait=[w], on_update=[])
                    new_insts.append(nop)
                si.on_wait = keep
            new_insts.append(ins)
        bb.instructions[:] = new_insts


def _host_prep(pose, image, W1, b1, W2, b2, src, dst):
    """Sort/pad edges by dst window, build per-core shards and onehot blocks."""
    E = pose.shape[0]
    Nn, C, H, Wsp = image.shape
    HW = H * Wsp
    CS = C // N_CORES
    F = CS * HW
    n_win = Nn // P

    src = np.asarray(src).astype(np.int64)
    dst = np.asarray(dst).astype(np.int64)

    order = np.argsort(dst, kind="stable")
    blk_edge = []  # [B, 128] edge id, -1 = pad
    blk_win = []
    for w in range(n_win):
        sel = order[(dst[order] >= w * P) & (dst[order] < (w + 1) * P)]
        nb = max(1, -(-len(sel) // P))
        for b in range(nb):
            seg = sel[b * P : (b + 1) * P]
            row = np.full(P, -1, np.int64)
            row[: len(seg)] = seg
            blk_edge.append(row)
            blk_win.append(w)
    while len(blk_edge) % BPC != 0:
        blk_edge.append(np.full(P, -1, np.int64))
        blk_win.append(n_win - 1)
    blk_edge = np.stack(blk_edge)  # [B, 128]
    B = len(blk_edge)
    Ep = B * P

    valid = blk_edge >= 0
    eids = np.where(valid, blk_edge, 0)

    # gather src per edge slot (pad -> node 0)
    blk_src = np.where(valid, src[eids], 0).astype(np.int32)  # [B,128]
    # onehot: [B, 128 edge, 128 local-node], zero row for pads
    loc = (np.where(valid, dst[eids], 0) - np.asarray(blk_win)[:, None] * P).astype(
        np.int64
    )
    oh = np.zeros((B, P, P), np.float32)
    bi, pi = np.nonzero(valid)
    oh[bi, pi, loc[bi, pi]] = 1.0
    oh = oh.astype(BF16)

    # poseT padded: [9, Ep]
    pose_pad = np.where(valid.reshape(-1, 1), pose[eids.reshape(-1)], 0.0)
    poseT = np.ascontiguousarray(pose_pad.T.astype(BF16))  # [9, Ep]

    b1_allzero = not np.any(b1)
    b2_allzero = not np.any(b2)
    cnt = np.bincount(dst, minlength=Nn).astype(np.float32)
    recip = np.where(cnt > 0, 1.0 / np.maximum(cnt, 1.0), 0.0).astype(np.float32)
    recip_t = np.ascontiguousarray(recip.reshape(n_win, P).T)  # [P, n_win]

    KT = C // P
    b1t = np.ascontiguousarray(b1.astype(np.float32).reshape(KT, P).T)  # [P, KT]
    idx_t = np.ascontiguousarray(blk_src.T)  # [P, B]

    # a block is skippable if it has no real edges and its window has other
    # real work (zero onehot rows contribute nothing)
    n_real = valid.sum(axis=1)
    win_blocks = {}
    for b, w in enumerate(blk_win):
        win_blocks.setdefault(w, []).append(b)
    keep = []
    for w, bs in win_blocks.items():
        real = [b for b in bs if n_real[b] > 0]
        keep.extend(real if real else bs[:1])
    keep = set(keep)
    # first/last block per window (among kept blocks)
    first_blk = {}
    last_blk = {}
    for b, w in enumerate(blk_win):
        if b in keep:
            first_blk.setdefault(w, b)
            last_blk[w] = b

    shared = dict(
        poseT=poseT,
        w1=W1.astype(BF16),
        b1t=b1t,
        idx=idx_t,
        oh=oh,
        recip=recip_t,
    )
    in_maps = []
    for j in range(N_CORES):
        c0 = j * CS
        cols_g = [2 * (c0 + i) for i in range(CS)]
        cols_b = [2 * (c0 + i) + 1 for i in range(CS)]
        cols = cols_b + cols_g  # [beta | gamma]
        w2f8 = W2[:, cols].astype(FP8)  # [C, 2*CS]
        b2row = b2[cols].reshape(1, -1).astype(BF16)  # [1, 2*CS]
        img = (
            image[:, c0 : c0 + CS]
            .transpose(0, 2, 3, 1)
            .reshape(Nn, F)
            .astype(BF16)
        )  # [Nn, F] layout [n, hw, c]
        in_maps.append(dict(shared, w2f8=w2f8, b2row=b2row, image=img))

    meta = dict(
        E=E, Nn=Nn, C=C, HW=HW, CS=CS, F=F, n_win=n_win, B=B, Ep=Ep, KT=KT,
        blk_win=blk_win, first_blk=first_blk, last_blk=last_blk,
        b1_allzero=b1_allzero, b2_allzero=b2_allzero, keep=keep,
    )
    return in_maps, meta


def _build(meta):
    Nn, CS, F, HW = meta["Nn"], meta["CS"], meta["F"], meta["HW"]
    n_win, B, Ep, KT = meta["n_win"], meta["B"], meta["Ep"], meta["KT"]
    C = meta["C"]
    blk_win, first_blk, last_blk = (
        meta["blk_win"], meta["first_blk"], meta["last_blk"],
    )
    f32 = mybir.dt.float32
    bf16 = mybir.dt.bfloat16
    fp8 = mybir.dt.float8e4
    NCH = B // BPC
    FB = F + CS  # messages + beta columns
    SEGW = 512
    seg_cols = [(s, min(SEGW, FB - s)) for s in range(0, FB, SEGW)]
    assert KT % 2 == 0, "fp8 DoubleRow path needs an even k-tile count"

    nc = bass.Bass()
    poseT_d = nc.declare_dram_parameter("poseT", [9, Ep], bf16, isOutput=False)
    w1_d = nc.declare_dram_parameter("w1", [9, C], bf16, isOutput=False)
    b1t_d = nc.declare_dram_parameter("b1t", [P, KT], f32, isOutput=False)
    w2f8_d = nc.declare_dram_parameter("w2f8", [C, 2 * CS], fp8, isOutput=False)
    b2_d = nc.declare_dram_parameter("b2row", [1, 2 * CS], bf16, isOutput=False)
    img_d = nc.declare_dram_parameter("image", [Nn, F], bf16, isOutput=False)
    idx_d = nc.declare_dram_parameter("idx", [P, B], mybir.dt.int32, isOutput=False)
    oh_d = nc.declare_dram_parameter("oh", [B, P, P], bf16, isOutput=False)
    recip_d = nc.declare_dram_parameter("recip", [P, n_win], f32, isOutput=False)
    out_d = nc.declare_dram_parameter("out", [Nn, F], bf16, isOutput=True)

    # chunk -> kept blocks
    chunk_blocks = []
    for ci in range(NCH):
        bs = [ci * BPC + bi for bi in range(BPC) if ci * BPC + bi in meta["keep"]]
        chunk_blocks.append(bs)

    with TileContext(nc) as tc:
        with (
            tc.tile_pool(name="const", bufs=1) as constp,
            tc.tile_pool(name="ht", bufs=3) as htp,
            tc.tile_pool(name="xg", bufs=4) as xp,
            tc.tile_pool(name="mm", bufs=2 * BPC + 2) as mp,
            tc.tile_pool(name="ohp", bufs=12) as ohp,
            tc.tile_pool(name="outp", bufs=3) as outp,
            tc.tile_pool(name="pse", bufs=2, space="PSUM") as psep,
            tc.tile_pool(name="pw", bufs=1, space="PSUM") as pwp,
        ):
            # ---- preload constants (idx first so gathers can start; first
            # chunk's pose slice on a second queue so PE starts immediately)
            idx_sb = constp.tile([P, B], mybir.dt.int32)
            nc.sync.dma_start(out=idx_sb[:], in_=idx_d[:])
            poseT_sb = constp.tile([9, Ep], bf16)
            nc.scalar.dma_start(
                out=poseT_sb[:, :CH_EDGES], in_=poseT_d[:, :CH_EDGES]
            )
            w1_sb = constp.tile([9, C], bf16)
            nc.sync.dma_start(out=w1_sb[:], in_=w1_d[:])
            b1_sb = constp.tile([P, KT], f32)
            nc.sync.dma_start(out=b1_sb[:], in_=b1t_d[:])
            w2_sb = constp.tile([P, KT * 2 * CS], fp8)
            for t in range(KT):
                nc.sync.dma_start(
                    out=w2_sb[:, t * 2 * CS : (t + 1) * 2 * CS],
                    in_=w2f8_d[t * P : (t + 1) * P, :],
                )
            recip_sb = constp.tile([P, n_win], f32)
            nc.sync.dma_start(out=recip_sb[:], in_=recip_d[:])
            nc.sync.dma_start(
                out=poseT_sb[:, CH_EDGES:], in_=poseT_d[:, CH_EDGES:]
            )
            b2_sb = constp.tile([1, 2 * CS], bf16)
            ones_sb = constp.tile([1, P], bf16)
            if not meta["b2_allzero"]:
                nc.sync.dma_start(out=b2_sb[:], in_=b2_d[:])
                nc.gpsimd.memset(ones_sb[:], 1.0)

            w23 = w2_sb.rearrange("p (t c) -> p t c", t=KT)

            # pipeline state carried across steps
            hT_of = {}     # ci -> hT tile
            Xg_of = {}     # ci -> gathered X tile [128, BPC*F]
            m_of = {}      # b -> message tile
            psw = [None]
            hev = [0]      # h-evac rotation counter

            def issue_gather(ci):
                if not chunk_blocks[ci]:
                    return
                Xg = xp.tile([P, BPC * F], bf16, tag="xg")
                Xg3 = Xg.rearrange("p (j f) -> p j f", j=BPC)
                for j in range(BPC):
                    nc.gpsimd.indirect_dma_start(
                        out=Xg3[:, j],
                        out_offset=None,
                        in_=img_d[:],
                        in_offset=bass.IndirectOffsetOnAxis(
                            ap=idx_sb[:, ci * BPC + j : ci * BPC + j + 1], axis=0
                        ),
                    )
                Xg_of[ci] = Xg

            def h_units(ci):
                if ci >= NCH or not chunk_blocks[ci]:
                    return []
                hT = htp.tile([P, KT * CH_EDGES], fp8, tag="ht")
                hT_of[ci] = hT
                e0 = ci * CH_EDGES
                units = []
                for t in range(KT):
                    units.append(lambda t=t, hT=hT, e0=e0: h_one(hT, e0, t))
                return units

            def h_one(hT, e0, t):
                if True:
                    ph = psep.tile([P, 512], f32, tag="ps")
                    nc.tensor.matmul(
                        out=ph[:, :CH_EDGES],
                        lhsT=w1_sb[:, t * P : (t + 1) * P],
                        rhs=poseT_sb[:, e0 : e0 + CH_EDGES],
                        start=True,
                        stop=True,
                    )
                    dst = hT[:, t * CH_EDGES : (t + 1) * CH_EDGES]
                    r = hev[0] % 9
                    hev[0] += 1
                    if r != 8:
                        nc.scalar.activation(
                            dst,
                            ph[:, :CH_EDGES],
                            mybir.ActivationFunctionType.Relu,
                            bias=b1_sb[:, t : t + 1],
                            scale=1.0,
                        )
                    else:
                        if meta["b1_allzero"]:
                            nc.vector.tensor_scalar_max(dst, ph[:, :CH_EDGES], 0.0)
                        else:
                            nc.vector.tensor_scalar(
                                out=dst, in0=ph[:, :CH_EDGES],
                                scalar1=b1_sb[:, t : t + 1], scalar2=0.0,
                                op0=mybir.AluOpType.add,
                                op1=mybir.AluOpType.max,
                            )

            def e_units(ci):
                if ci < 0:
                    return []
                hT = hT_of.get(ci)
                if hT is None:
                    return []
                hT3 = hT.rearrange("p (t e) -> p t e", t=KT)
                return [
                    lambda b=b, hT3=hT3, ci=ci: e_one(hT3, ci, b)
                    for b in chunk_blocks[ci]
                ]

            def e_one(hT3, ci, b):
                if True:
                    bi = b - ci * BPC
                    pe_ps = psep.tile([P, 512], f32, tag="ps")
                    for t2 in range(KT // 2):
                        nc.tensor.matmul(
                            out=pe_ps[:, : 2 * CS],
                            lhsT=hT3[:, 2 * t2 : 2 * t2 + 2, bi * P : (bi + 1) * P],
                            rhs=w23[:, 2 * t2 : 2 * t2 + 2, :],
                            start=(t2 == 0),
                            stop=(t2 == KT // 2 - 1 and meta["b2_allzero"]),
                            perf_mode=mybir.MatmulPerfMode.DoubleRow,
                        )
                    if not meta["b2_allzero"]:
                        nc.tensor.matmul(
                            out=pe_ps[:, : 2 * CS],
                            lhsT=ones_sb[:1, :P],
                            rhs=b2_sb[:1, :],
                            start=False,
                            stop=True,
                        )
                    m = mp.tile([P, F + 2 * CS], bf16, tag="mm", name="m")
                    m_of[b] = m
                    nc.scalar.activation(
                        m[:, F : F + 2 * CS], pe_ps[:, : 2 * CS],
                        mybir.ActivationFunctionType.Sigmoid,
                    )

            def emit_mult(ci):
                Xg = Xg_of.get(ci)
                if Xg is None:
                    return
                Xg4 = Xg.rearrange("p (j o c) -> p j o c", j=BPC, o=HW)
                for b in chunk_blocks[ci]:
                    bi = b - ci * BPC
                    m = m_of[b]
                    g_b1 = m[:, FB : FB + CS].rearrange("p (o c) -> p o c", o=1)
                    m3 = m[:, :F].rearrange("p (o c) -> p o c", o=HW)
                    nc.vector.tensor_tensor(
                        out=m3[:],
                        in0=Xg4[:, bi],
                        in1=g_b1.to_broadcast([P, HW, CS]),
                        op=mybir.AluOpType.mult,
                    )

            def scatter_units(ci):
                if ci < 0:
                    return []
                return [lambda b=b: scatter_one(b) for b in chunk_blocks[ci]]

            def scatter_one(b):
                if True:
                    w = blk_win[b]
                    m = m_of.pop(b)
                    oht = ohp.tile([P, P], bf16, tag="oh")
                    nc.sync.dma_start(out=oht[:], in_=oh_d[b])
                    first = first_blk[w] == b
                    last = last_blk[w] == b
                    if first:
                        psw[0] = pwp.tile([P, FB], f32, tag="pw", name="psw")
                    for s, width in seg_cols:
                        nc.tensor.matmul(
                            out=psw[0][:, s : s + width],
                            lhsT=oht[:],
                            rhs=m[:, s : s + width],
                            start=first,
                            stop=last,
                            skip_group_check=True,
                        )
                    if last:
                        emit_evac(psw[0], w)

            def emit_evac(pw_t, w):
                # out = psum*recip + (beta_sum*recip) broadcast over hw
                bs = outp.tile([P, CS], bf16, tag="bs")
                nc.scalar.activation(
                    bs[:], pw_t[:, F:FB],
                    mybir.ActivationFunctionType.Copy,
                    scale=recip_sb[:, w : w + 1],
                )
                of = outp.tile([P, F], bf16, tag="of")
                of3 = of.rearrange("p (o c) -> p o c", o=HW)
                pw3 = pw_t[:, :F].rearrange("p (o c) -> p o c", o=HW)
                bs_b = bs.rearrange("p (o c) -> p o c", o=1)
                nc.vector.scalar_tensor_tensor(
                    out=of3[:],
                    in0=pw3[:],
                    scalar=recip_sb[:, w : w + 1],
                    in1=bs_b.to_broadcast([P, HW, CS]),
                    op0=mybir.AluOpType.mult,
                    op1=mybir.AluOpType.add,
                )
                nc.sync.dma_start(out=out_d[w * P : (w + 1) * P, :], in_=of[:])

            # ---- software-pipelined emission: 2-chunk skew
            for ci in range(min(3, NCH)):
                issue_gather(ci)
            for i in range(NCH + 2):
                hu = h_units(i)
                eu = e_units(i - 1)
                su = scatter_units(i - 2)
                # weave h matmuls between e/scatter blocks so the PE stream
                # never bursts h (whose PSUM evac would pace it) and never
                # idles: pattern e,h,h,e,h,h,... then s,h,s,h,...
                blocks = eu + su
                nh, nb = len(hu), len(blocks)
                out_units = []
                hi = 0
                for k, bu in enumerate(blocks):
                    out_units.append(bu)
                    take = ((k + 1) * nh) // (nb or 1) - hi
                    for _ in range(take):
                        out_units.append(hu[hi]); hi += 1
                out_units.extend(hu[hi:])
                for u in out_units:
                    u()
                if 0 <= i - 1:
                    emit_mult(i - 1)
                if i + 3 < NCH:
                    issue_gather(i + 3)

    _split_excess_waits(nc)
    return nc


def _run(inputs, trace=False, trace_kwargs=None):
    pose = np.asarray(inputs["pose"], np.float32)
    image = np.asarray(inputs["image"], np.float32)
    W1 = np.asarray(inputs["W1"], np.float32)
    b1 = np.asarray(inputs["b1"], np.float32)
    W2 = np.asarray(inputs["W2"], np.float32)
    b2 = np.asarray(inputs["b2"], np.float32)
    src = np.asarray(inputs["src"])
    dst = np.asarray(inputs["dst"])

    in_maps, meta = _host_prep(pose, image, W1, b1, W2, b2, src, dst)
    nc = _build(meta)
    kw = {}
    if trace:
        kw = dict(trace=True, trace_kwargs=trace_kwargs or {})
    res = run_bass_kernel_spmd(nc, in_maps, core_ids=list(range(N_CORES)), **kw)
    Nn, C, HW, CS = meta["Nn"], meta["C"], meta["HW"], meta["CS"]
    H = int(np.sqrt(HW))
    out = np.empty((Nn, C, H, HW // H), np.float32)
    for j in range(N_CORES):
        oc = np.asarray(res.results[j]["out"]).astype(np.float32)
        out[:, j * CS : (j + 1) * CS] = (
            oc.reshape(Nn, HW, CS).transpose(0, 2, 1).reshape(Nn, CS, H, HW // H)
        )
    return out, res


def kernel(**inputs) -> np.ndarray:
    out, _ = _run(inputs)
    return out
